# revision 31
# baseline (speedup 1.0000x reference)
"""Context-attention kernel for 8 TRN2 NeuronCores.

Reference math (T=100000 tokens, H=256):
    proj   = hc @ W1.T                          # [1, H]
    At     = tanh(Mt @ Wm + proj) @ V           # [T, 1]
    alphat = log_softmax(At.T)                  # [1, T]
    ct     = alphat @ Mt                        # [1, H]

Key restructuring:
  * ct is linear in alphat = At - logZ, so ct = u - logZ * s with
    u = sum_t At_t * Mt[t,:] and s = sum_t Mt[t,:].  Each core computes u over
    its token shard in the same single pass that produces At; the scalar logZ
    correction happens on host from the raw At values.
  * Tokens are sharded 12500/core, padded to 12544 = 24x512 + 256 chunk cols.
  * On-chip layout is channels-first: the host stages Mt.T per core in bf16
    ([H, 12544], zero-padded).  That halves HBM traffic vs f32 (the problem is
    memory-regime) and puts hidden dims on partitions so:
      - Y.T = Wm.T @ Mt.T runs with natural-layout Wm quarters as stationary
        operands (no transposes anywhere),
      - the +proj bias is per-partition, folded into the tanh activation,
      - At arrives broadcast across all 128 partitions by using a host-staged
        column-replicated V as the stationary operand, which is exactly the
        shape the DVE affine_mul_reduce needs to accumulate u.
  * s is exact column sums, computed on host in f64 (part of unsharding).
"""

import numpy as np
import ml_dtypes

T = 100000
H = 256
NCORES = 8
TC = T // NCORES          # 12500 tokens per core
CH = 512                  # chunk: tokens per PSUM tile
NCH = 25                  # chunks per core (last one is 256 wide)
TP = 12544                # padded tokens per core = 24*512 + 256
# Growing DMA superchunks (tokens): small first so compute starts early,
# large later for HBM efficiency.  Sums to TP, all multiples of CH.
SC_SIZES = (512, 1024, 2048, 3072, 5888)

_BF16 = ml_dtypes.bfloat16
_BUILT = None
_last_in_maps = None


def _chunk_cols(c):
    col0 = c * CH
    w = min(CH, TP - col0)
    return col0, w


def _build(stage=5):
    import concourse.bacc as bacc
    import concourse.mybir as mybir
    from concourse.tile import TileContext

    dt = mybir.dt
    AF = mybir.ActivationFunctionType
    ALU = mybir.AluOpType

    nc = bacc.Bacc()
    mtT = nc.declare_dram_parameter("mtT", [H, TP], dt.bfloat16, isOutput=False)
    # Packed constants: [Wm (256 j) | V replicated (128) | proj (1)] per k/j row.
    wmv = nc.declare_dram_parameter("wmv", [H, 385], dt.bfloat16, isOutput=False)
    at_out = nc.declare_dram_parameter("at_out", [1, TP], dt.bfloat16, isOutput=True)
    u_out = nc.declare_dram_parameter("u_out", [2, 128, 1], dt.float32, isOutput=True)

    sc_bounds = []
    lo = 0
    for sz in SC_SIZES:
        sc_bounds.append((lo, lo + sz))
        lo += sz
    assert lo == TP

    with TileContext(nc) as tc:
        with (
            tc.sbuf_pool(name="const", bufs=1) as cpool,
            tc.sbuf_pool(name="mt", bufs=1) as mtpool,
            tc.sbuf_pool(name="work", bufs=3) as wpool,
            tc.sbuf_pool(name="accum", bufs=1) as apool,
            tc.psum_pool(name="psY", bufs=3) as psY,
            tc.psum_pool(name="psA", bufs=2) as psA,
        ):
            wmv_sb = []
            for h in range(2):
                w = cpool.tile([128, 385], dt.bfloat16, name=f"wmv_sb{h}")
                eng = nc.sync if h == 0 else nc.gpsimd
                eng.dma_start(out=w[:], in_=wmv[h * 128 : (h + 1) * 128, :])
                wmv_sb.append(w)

            mt_sb = {}
            for si, (slo, shi) in enumerate(sc_bounds):
                for kh in range(2):
                    t = mtpool.tile(
                        [128, shi - slo], dt.bfloat16, name=f"mt_sb{kh}_{si}",
                        tag=f"mt_sb{kh}_{si}",
                    )
                    eng = nc.sync if kh == 0 else nc.gpsimd
                    eng.dma_start(
                        out=t[:], in_=mtT[kh * 128 : (kh + 1) * 128, slo:shi]
                    )
                    mt_sb[kh, si] = t

            at_sb = apool.tile([128, TP], dt.bfloat16, name="at_sb")
            # Per-chunk u partials land in their own column; one reduce at the
            # end collapses them.  No serial chain between chunks.
            u_part = [
                apool.tile([128, NCH], dt.float32, name=f"u_part{kh}")
                for kh in range(2)
            ]
            u_fin = [
                apool.tile([128, 1], dt.float32, name=f"u_fin{kh}") for kh in range(2)
            ]

            for c in range(NCH):
                col0, w = _chunk_cols(c)
                si = next(i for i, (slo, shi) in enumerate(sc_bounds) if col0 < shi)
                off = col0 - sc_bounds[si][0]

                if stage < 2:
                    nc.vector.tensor_copy(
                        at_sb[:, col0 : col0 + w], mt_sb[0, si][:, off : off + w]
                    )
                    continue
                yt = []
                for jh in range(2):
                    y = psY.tile(
                        [128, w], dt.float32, name=f"yt{jh}", tag=f"yt{jh}",
                        padded_shape=[128, CH],
                    )
                    for kh in range(2):
                        nc.tensor.matmul(
                            y[:],
                            wmv_sb[kh][:, jh * 128 : (jh + 1) * 128],
                            mt_sb[kh, si][:, off : off + w],
                            start=(kh == 0),
                            stop=(kh == 1),
                        )
                    yt.append(y)

                if stage < 3:
                    nc.scalar.copy(at_sb[:, col0 : col0 + w], yt[0][:])
                    continue
                tnh = []
                for jh in range(2):
                    t = wpool.tile(
                        [128, w], dt.bfloat16, name=f"tnh{jh}", tag=f"tnh{jh}",
                        padded_shape=[128, CH],
                    )
                    nc.scalar.activation(
                        t[:], yt[jh][:], AF.Tanh,
                        bias=wmv_sb[jh][:, 384:385], scale=1.0,
                    )
                    tnh.append(t)

                if stage < 4:
                    nc.vector.tensor_copy(at_sb[:, col0 : col0 + w], tnh[0][:])
                    continue
                pat = psA.tile(
                    [128, w], dt.float32, name="pat", tag="pat",
                    padded_shape=[128, CH],
                )
                for jh in range(2):
                    nc.tensor.matmul(
                        pat[:], wmv_sb[jh][:, 256:384], tnh[jh][:],
                        start=(jh == 0), stop=(jh == 1),
                    )
                # PSUM -> SBUF evacuation alternates engines to balance load.
                if c % 2 == 0:
                    nc.scalar.copy(at_sb[:, col0 : col0 + w], pat[:])
                else:
                    nc.vector.tensor_copy(at_sb[:, col0 : col0 + w], pat[:])

                if stage < 5:
                    continue
                for kh in range(2):
                    scr = wpool.tile(
                        [128, w], dt.bfloat16, name="scr", tag=f"scr{kh}",
                        padded_shape=[128, CH],
                    )
                    nc.vector.affine_mul_reduce(
                        out=scr[:],
                        accum_out=u_part[kh][:, c : c + 1],
                        in0=mt_sb[kh, si][:, off : off + w],
                        in1=at_sb[:, col0 : col0 + w],
                        scale=1.0,
                        bias=0.0,
                    )

            nc.sync.dma_start(out=at_out[0:1, :], in_=at_sb[0:1, :])
            for kh in range(2):
                if stage >= 5:
                    nc.vector.tensor_reduce(
                        u_fin[kh][:], u_part[kh][:], mybir.AxisListType.X, ALU.add,
                    )
                    src = u_fin[kh]
                else:
                    src = wmv_sb[0][:, 0:1]
                nc.sync.dma_start(out=u_out[kh], in_=src[:])

    nc.finalize()
    return nc


def _get_built():
    global _BUILT
    if _BUILT is None:
        _BUILT = _build()
    return _BUILT


def kernel(inputs, hc, Wm, V, W1):
    from concourse.bass_utils import run_bass_kernel_spmd

    inputs = np.asarray(inputs, dtype=np.float32)
    hc = np.asarray(hc, dtype=np.float32)
    Wm = np.asarray(Wm, dtype=np.float32)
    V = np.asarray(V, dtype=np.float32)
    W1 = np.asarray(W1, dtype=np.float32)

    proj = (hc @ W1.T).reshape(H).astype(np.float32)
    wmv_np = np.zeros((H, 385), dtype=_BF16)
    wmv_np[:, :256] = Wm.astype(_BF16)
    wmv_np[:, 256:384] = np.tile(V.astype(_BF16), (1, 128))
    wmv_np[:, 384] = proj.astype(_BF16)
    s_host = inputs.sum(axis=0, dtype=np.float64).astype(np.float32)

    in_maps = []
    for c in range(NCORES):
        shard = inputs[c * TC : (c + 1) * TC]
        mtT_np = np.zeros((H, TP), dtype=_BF16)
        mtT_np[:, :TC] = shard.astype(_BF16).T
        in_maps.append({"mtT": mtT_np, "wmv": wmv_np})

    global _last_in_maps
    _last_in_maps = in_maps
    nc = _get_built()
    res = run_bass_kernel_spmd(nc, in_maps, core_ids=list(range(NCORES)))
    outs = res.results

    at_parts = []
    u_tot = np.zeros(H, dtype=np.float64)
    for c in range(NCORES):
        at_c = np.asarray(outs[c]["at_out"]).reshape(TP)[:TC].astype(np.float32)
        at_parts.append(at_c)
        u_c = np.asarray(outs[c]["u_out"]).reshape(2, 128)
        u_tot += u_c.reshape(H).astype(np.float64)

    at_full = np.concatenate(at_parts)
    m = float(at_full.max())
    z = float(np.exp(at_full - m, dtype=np.float32).sum(dtype=np.float64))
    log_z = np.float32(m + np.log(z))
    alphat = (at_full - log_z).astype(np.float32).reshape(1, T)
    ct = (u_tot.astype(np.float32) - log_z * s_host).reshape(1, H)
    return (alphat, ct)


# revision 62
# speedup vs baseline: 1.0345x; 1.0345x over previous
"""Context-attention kernel for 8 TRN2 NeuronCores.

Reference math (T=100000 tokens, H=256):
    proj   = hc @ W1.T                          # [1, H]
    At     = tanh(Mt @ Wm + proj) @ V           # [T, 1]
    alphat = log_softmax(At.T)                  # [1, T]
    ct     = alphat @ Mt                        # [1, H]

Key restructuring:
  * ct is linear in alphat = At - logZ, so ct = u - logZ * s with
    u = sum_t At_t * Mt[t,:] and s = sum_t Mt[t,:].  Each core computes u over
    its token shard in the same single pass that produces At; the scalar logZ
    correction happens on host from the raw At values.
  * Tokens are sharded 12500/core, padded to 12544 = 24x512 + 256 chunk cols.
  * On-chip layout is channels-first: the host stages Mt.T per core in bf16
    ([H, 12544], zero-padded).  That halves HBM traffic vs f32 (the problem is
    memory-regime) and puts hidden dims on partitions so:
      - Y.T = Wm.T @ Mt.T runs with natural-layout Wm quarters as stationary
        operands (no transposes anywhere),
      - the +proj bias is per-partition, folded into the tanh activation,
      - At arrives broadcast across all 128 partitions by using a host-staged
        column-replicated V as the stationary operand, which is exactly the
        shape the DVE affine_mul_reduce needs to accumulate u.
  * s is exact column sums, computed on host in f64 (part of unsharding).
"""

import numpy as np
import ml_dtypes

T = 100000
H = 256
NCORES = 8
TC = T // NCORES          # 12500 tokens per core
CH = 512                  # chunk: tokens per PSUM tile
NCH = 25                  # chunks per core (last one is 256 wide)
TP = 12544                # padded tokens per core = 24*512 + 256
# Growing DMA superchunks (tokens): small first so compute starts early,
# large later for HBM efficiency.  Sums to TP, all multiples of CH.
SC_SIZES = (1536, 2048, 3072, 5888)

_BF16 = ml_dtypes.bfloat16
_BUILT = None
_last_in_maps = None


def _chunk_cols(c):
    col0 = c * CH
    w = min(CH, TP - col0)
    return col0, w


def _build(stage=5):
    import concourse.bacc as bacc
    import concourse.mybir as mybir
    from concourse.tile import TileContext

    dt = mybir.dt
    AF = mybir.ActivationFunctionType
    ALU = mybir.AluOpType

    nc = bacc.Bacc()
    mtT = nc.declare_dram_parameter("mtT", [H, TP], dt.bfloat16, isOutput=False)
    # Packed constants: [Wm (256 j) | V replicated (128) | proj (1)] per k/j row.
    wmv = nc.declare_dram_parameter("wmv", [H, 385], dt.bfloat16, isOutput=False)
    at_out = nc.declare_dram_parameter("at_out", [1, TP], dt.bfloat16, isOutput=True)
    u_out = nc.declare_dram_parameter("u_out", [128, 2], dt.float32, isOutput=True)

    sc_bounds = []
    lo = 0
    for sz in SC_SIZES:
        sc_bounds.append((lo, lo + sz))
        lo += sz
    assert lo == TP

    with TileContext(nc) as tc:
        with (
            tc.sbuf_pool(name="const", bufs=1) as cpool,
            tc.sbuf_pool(name="mt", bufs=1) as mtpool,
            tc.sbuf_pool(name="work", bufs=3) as wpool,
            tc.sbuf_pool(name="accum", bufs=1) as apool,
            tc.psum_pool(name="psY", bufs=3) as psY,
            tc.psum_pool(name="psA", bufs=2) as psA,
        ):
            wmv_sb = []
            for h in range(2):
                w = cpool.tile([128, 385], dt.bfloat16, name=f"wmv_sb{h}")
                eng = nc.sync if h == 0 else nc.gpsimd
                eng.dma_start(out=w[:], in_=wmv[h * 128 : (h + 1) * 128, :])
                wmv_sb.append(w)

            mt_sb = {}
            for si, (slo, shi) in enumerate(sc_bounds):
                for kh in range(2):
                    t = mtpool.tile(
                        [128, shi - slo], dt.bfloat16, name=f"mt_sb{kh}_{si}",
                        tag=f"mt_sb{kh}_{si}",
                    )
                    eng = nc.sync if kh == 0 else nc.gpsimd
                    eng.dma_start(
                        out=t[:], in_=mtT[kh * 128 : (kh + 1) * 128, slo:shi]
                    )
                    mt_sb[kh, si] = t

            at_sb = apool.tile([128, TP], dt.bfloat16, name="at_sb")
            # Per-chunk u partials land in their own column; one reduce at the
            # end collapses them.  No serial chain between chunks.
            u_part = [
                apool.tile([128, NCH], dt.float32, name=f"u_part{kh}")
                for kh in range(2)
            ]
            u_fin = apool.tile([128, 2], dt.float32, name="u_fin")

            for c in range(NCH):
                col0, w = _chunk_cols(c)
                si = next(i for i, (slo, shi) in enumerate(sc_bounds) if col0 < shi)
                off = col0 - sc_bounds[si][0]

                if stage < 2:
                    nc.vector.tensor_copy(
                        at_sb[:, col0 : col0 + w], mt_sb[0, si][:, off : off + w]
                    )
                    continue
                yt = []
                for jh in range(2):
                    y = psY.tile(
                        [128, w], dt.float32, name=f"yt{jh}", tag=f"yt{jh}",
                        padded_shape=[128, CH], bufs=2 if jh == 0 else 3,
                    )
                    for kh in range(2):
                        nc.tensor.matmul(
                            y[:],
                            wmv_sb[kh][:, jh * 128 : (jh + 1) * 128],
                            mt_sb[kh, si][:, off : off + w],
                            start=(kh == 0),
                            stop=(kh == 1),
                        )
                    yt.append(y)

                if stage < 3:
                    nc.scalar.copy(at_sb[:, col0 : col0 + w], yt[0][:])
                    continue
                tnh = []
                for jh in range(2):
                    t = wpool.tile(
                        [128, w], dt.bfloat16, name=f"tnh{jh}", tag=f"tnh{jh}",
                        padded_shape=[128, CH],
                    )
                    nc.scalar.activation(
                        t[:], yt[jh][:], AF.Tanh,
                        bias=wmv_sb[jh][:, 384:385], scale=1.0,
                    )
                    tnh.append(t)

                if stage < 4:
                    nc.vector.tensor_copy(at_sb[:, col0 : col0 + w], tnh[0][:])
                    continue
                pat = psA.tile(
                    [128, w], dt.float32, name="pat", tag="pat",
                    padded_shape=[128, CH], bufs=3,
                )
                for jh in range(2):
                    nc.tensor.matmul(
                        pat[:], wmv_sb[jh][:, 256:384], tnh[jh][:],
                        start=(jh == 0), stop=(jh == 1),
                    )
                # PSUM -> SBUF evacuation: DVE takes the early chunks (it
                # idles during pipeline fill) plus odd chunks; ScalarE (the
                # critical engine) keeps only 10 late evacs.
                if c >= 18 or (c >= 6 and c % 2 == 0):
                    nc.scalar.copy(at_sb[:, col0 : col0 + w], pat[:])
                else:
                    nc.vector.tensor_copy(at_sb[:, col0 : col0 + w], pat[:])

                if stage < 5:
                    continue
                for kh in range(2):
                    scr = wpool.tile(
                        [128, w], dt.bfloat16, name="scr", tag=f"scr{kh}",
                        padded_shape=[128, CH],
                    )
                    nc.vector.affine_mul_reduce(
                        out=scr[:],
                        accum_out=u_part[kh][:, c : c + 1],
                        in0=mt_sb[kh, si][:, off : off + w],
                        in1=at_sb[:, col0 : col0 + w],
                        scale=1.0,
                        bias=0.0,
                    )

            half = (NCH // 2) * CH
            nc.sync.dma_start(out=at_out[0:1, :half], in_=at_sb[0:1, :half])
            nc.gpsimd.dma_start(out=at_out[0:1, half:], in_=at_sb[0:1, half:])
            if stage >= 5:
                for kh in range(2):
                    nc.vector.tensor_reduce(
                        u_fin[:, kh : kh + 1], u_part[kh][:],
                        mybir.AxisListType.X, ALU.add,
                    )
                nc.sync.dma_start(out=u_out[:], in_=u_fin[:])
            else:
                nc.sync.dma_start(out=u_out[:], in_=wmv_sb[0][:, 0:2])

    nc.finalize()
    return nc


def _get_built():
    global _BUILT
    if _BUILT is None:
        _BUILT = _build()
    return _BUILT


def kernel(inputs, hc, Wm, V, W1):
    from concourse.bass_utils import run_bass_kernel_spmd

    inputs = np.asarray(inputs, dtype=np.float32)
    hc = np.asarray(hc, dtype=np.float32)
    Wm = np.asarray(Wm, dtype=np.float32)
    V = np.asarray(V, dtype=np.float32)
    W1 = np.asarray(W1, dtype=np.float32)

    proj = (hc @ W1.T).reshape(H).astype(np.float32)
    wmv_np = np.zeros((H, 385), dtype=_BF16)
    wmv_np[:, :256] = Wm.astype(_BF16)
    wmv_np[:, 256:384] = np.tile(V.astype(_BF16), (1, 128))
    wmv_np[:, 384] = proj.astype(_BF16)
    s_host = inputs.sum(axis=0, dtype=np.float64).astype(np.float32)

    in_maps = []
    for c in range(NCORES):
        shard = inputs[c * TC : (c + 1) * TC]
        mtT_np = np.zeros((H, TP), dtype=_BF16)
        mtT_np[:, :TC] = shard.astype(_BF16).T
        in_maps.append({"mtT": mtT_np, "wmv": wmv_np})

    global _last_in_maps
    _last_in_maps = in_maps
    nc = _get_built()
    res = run_bass_kernel_spmd(nc, in_maps, core_ids=list(range(NCORES)))
    outs = res.results

    at_parts = []
    u_tot = np.zeros(H, dtype=np.float64)
    for c in range(NCORES):
        at_c = np.asarray(outs[c]["at_out"]).reshape(TP)[:TC].astype(np.float32)
        at_parts.append(at_c)
        u_c = np.asarray(outs[c]["u_out"]).reshape(128, 2)
        u_tot += u_c.T.reshape(H).astype(np.float64)

    at_full = np.concatenate(at_parts)
    m = float(at_full.max())
    z = float(np.exp(at_full - m, dtype=np.float32).sum(dtype=np.float64))
    log_z = np.float32(m + np.log(z))
    alphat = (at_full - log_z).astype(np.float32).reshape(1, T)
    ct = (u_tot.astype(np.float32) - log_z * s_host).reshape(1, H)
    return (alphat, ct)
